# revision 2
# baseline (speedup 1.0000x reference)
"""CPM3 attention kernel for 8 trn2 NeuronCores.

Sharding: tensor-parallel over heads (2 heads/core x both batches).
Device computes per-core partial outputs (Wo row-sharded); host sums.

Data layout tricks:
- host pre-transposes q/kv so the device never transposes big tensors;
  scores are computed transposed [k, q] so the softmax needs no
  partition-dim reductions (a ones-column in V yields the denominators).
- fp16 operands for all matmuls: 2-byte weights use the PE background
  weight-load path and halve HBM traffic. PSUM accumulation stays fp32.
- softmax bias/mask enter MULTIPLICATIVELY: host precomputes
  E = exp(position_bias) * keep_mask, so p = exp(qk) * E.  This removes
  the per-tile identity matmul (PE) and scalar_tensor_tensor combine
  (DVE) of the additive formulation; the DVE does one fast fp16
  tensor_tensor multiply per tile instead.
"""

import sys

sys.path.insert(0, "/opt/trn_rl_repo")

import numpy as np
import ml_dtypes

import concourse.bass as bass
import concourse.bacc as bacc
import concourse.tile as tile
import concourse.mybir as mybir
from concourse.bass_utils import run_bass_kernel_spmd

B, L, D, H, DH = 2, 2048, 1024, 16, 64
N_CORES = 8
HPC = H // N_CORES  # heads per core = 2
QTS = 512  # q tile size
QN = L // QTS  # 4
KP = 128  # k partition tile
KN = L // KP  # 16
KTG = 4  # k tiles per DMA group
KGN = KN // KTG  # 4
DC = D // 128  # 8 contraction chunks
HVW = 2 * (DH + 1)  # 130: hv_aug columns per k-tile (2 heads x (64+ones))

F32 = mybir.dt.float32
F32R = mybir.dt.float32r
F16 = mybir.dt.float16
U8 = mybir.dt.uint8

_CACHE: dict = {}


def _build():
    if "nc" in _CACHE:
        return _CACHE["nc"]
    nc = bacc.Bacc("TRN2", target_bir_lowering=False, debug=False, num_devices=N_CORES)

    qT = nc.dram_tensor("qT", [B, DC, 128, L], F16, kind="ExternalInput").ap()
    kvT = nc.dram_tensor("kvT", [B, DC, 128, L], F16, kind="ExternalInput").ap()
    wq = nc.dram_tensor("wq", [128, DC, 128], F16, kind="ExternalInput").ap()
    wk = nc.dram_tensor("wk", [128, DC, 128], F16, kind="ExternalInput").ap()
    wv = nc.dram_tensor("wv", [128, DC, 128], F16, kind="ExternalInput").ap()
    wo = nc.dram_tensor("wo", [128, D], F16, kind="ExternalInput").ap()
    eb = nc.dram_tensor(
        "eb", [QN, KGN, 128, KTG, B, HPC, QTS], F16, kind="ExternalInput"
    ).ap()
    identr = nc.dram_tensor("identr", [128, 128], F32R, kind="ExternalInput").ap()
    indh = nc.dram_tensor("indh", [1, 256], F16, kind="ExternalInput").ap()
    out = nc.dram_tensor("out", [B, L, D], F16, kind="ExternalOutput").ap()

    with tile.TileContext(nc) as tc:
        with (
            tc.tile_pool(name="const", bufs=1) as constp,
            tc.tile_pool(name="hq", bufs=2) as hqp,
            tc.tile_pool(name="hk", bufs=2) as hkp,
            tc.tile_pool(name="hv", bufs=2) as hvp,
            tc.tile_pool(name="stage", bufs=3) as stagep,
            tc.tile_pool(name="ebp", bufs=2) as ebp,
            tc.tile_pool(name="p1", bufs=12) as p1p,
            tc.tile_pool(name="pt", bufs=10) as ptp,
            tc.tile_pool(name="ctxn", bufs=2) as ctxnp,
            tc.tile_pool(name="rc", bufs=4) as rcp,
            tc.tile_pool(name="outb", bufs=3) as outp,
            tc.tile_pool(name="psum", bufs=8, space=bass.MemorySpace.PSUM) as psp,
        ):
            # ---- constants ----
            identr_t = constp.tile([128, 128], F32R, tag="identr")
            nc.sync.dma_start(identr_t[:], identr[:])
            indh_t = constp.tile([1, 256], F16, tag="indh")
            nc.sync.dma_start(indh_t[:], indh[:])
            wq_t = constp.tile([128, DC, 128], F16, tag="wq")
            nc.sync.dma_start(wq_t[:], wq[:])
            wk_t = constp.tile([128, DC, 128], F16, tag="wk")
            nc.sync.dma_start(wk_t[:], wk[:])
            wv_t = constp.tile([128, DC, 128], F16, tag="wv")
            nc.sync.dma_start(wv_t[:], wv[:])
            wo_t = constp.tile([128, D], F16, tag="wo")
            nc.sync.dma_start(wo_t[:], wo[:])

            # ---- prefetch first E group (overlaps prologue) ----
            pre_eb = ebp.tile([128, KTG, B, HPC, QTS], F16, tag="eb", name="pre_eb")
            nc.scalar.dma_start(pre_eb[:], eb[0, 0])

            # ---- prologue: projections ----
            hq_sb, hk_sb, hv_sb = {}, {}, {}
            for b in range(B):
                hq_sb[b] = hqp.tile([128, L], F16, tag="hq", name=f"hq_sb{b}")
                hq_ps = [
                    psp.tile([128, QTS], F32, tag="bank", name=f"hq_ps{b}_{i}")
                    for i in range(QN)
                ]
                for dc in range(DC):
                    qc = stagep.tile([128, L], F16, tag="stage")
                    nc.sync.dma_start(qc[:], qT[b, dc])
                    for qt in range(QN):
                        nc.tensor.matmul(
                            hq_ps[qt][:],
                            wq_t[:, dc, :],
                            qc[:, qt * QTS : (qt + 1) * QTS],
                            start=(dc == 0),
                            stop=(dc == DC - 1),
                        )
                for qt in range(QN):
                    nc.scalar.copy(
                        hq_sb[b][:, qt * QTS : (qt + 1) * QTS], hq_ps[qt][:]
                    )

                hk_sb[b] = hkp.tile([128, L], F16, tag="hk", name=f"hk_sb{b}")
                hvT = stagep.tile([128, L], F32R, tag="stage")
                hk_ps = [
                    psp.tile([128, QTS], F32, tag="bank", name=f"hk_ps{b}_{i}")
                    for i in range(QN)
                ]
                hv_ps = [
                    psp.tile([128, QTS], F32, tag="bank", name=f"hv_ps{b}_{i}")
                    for i in range(QN)
                ]
                for dc in range(DC):
                    kc = stagep.tile([128, L], F16, tag="stage")
                    nc.sync.dma_start(kc[:], kvT[b, dc])
                    for qt in range(QN):
                        nc.tensor.matmul(
                            hk_ps[qt][:],
                            wk_t[:, dc, :],
                            kc[:, qt * QTS : (qt + 1) * QTS],
                            start=(dc == 0),
                            stop=(dc == DC - 1),
                        )
                        nc.tensor.matmul(
                            hv_ps[qt][:],
                            wv_t[:, dc, :],
                            kc[:, qt * QTS : (qt + 1) * QTS],
                            start=(dc == 0),
                            stop=(dc == DC - 1),
                        )
                for qt in range(QN):
                    nc.scalar.copy(
                        hk_sb[b][:, qt * QTS : (qt + 1) * QTS], hk_ps[qt][:]
                    )
                    nc.vector.tensor_copy(
                        hvT[:, qt * QTS : (qt + 1) * QTS], hv_ps[qt][:]
                    )

                # hv_aug: transpose hvT per k-tile; ones cols prefilled
                hv_sb[b] = hvp.tile([128, KN * HVW + 64], F16, tag="hv", name=f"hv_sb{b}")
                nc.gpsimd.memset(hv_sb[b][:].bitcast(mybir.dt.uint16), 0x3C00)
                for kt in range(KN):
                    tp = psp.tile([128, 128], F32R, tag="bank")
                    nc.tensor.transpose(
                        tp[:], hvT[:, kt * KP : (kt + 1) * KP], identr_t[:]
                    )
                    o = kt * HVW
                    nc.vector.tensor_copy(hv_sb[b][:, o : o + DH], tp[:, 0:DH])
                    nc.vector.tensor_copy(
                        hv_sb[b][:, o + DH + 1 : o + 2 * DH + 1], tp[:, DH:128]
                    )

            # ---- main loop ----
            def emit_epilogue(qt, ctx_ps):
                # normalize + output projection for a finished q-tile
                for b in range(B):
                    ctxn = ctxnp.tile(
                        [128, QTS], F16, tag="ctxn", name=f"ctxn{b}_{qt}"
                    )
                    bcw = psp.tile([128, QTS], F32, tag="bank", name=f"bcw{b}_{qt}")
                    bc = bcw[:]
                    for h in range(HPC):
                        dsb = rcp.tile(
                            [1, QTS], F32, tag="dsb", name=f"dsb{b}_{h}_{qt}"
                        )
                        nc.vector.tensor_copy(dsb[:], ctx_ps[(b, h)][DH : DH + 1, :])
                        rcf = rcp.tile(
                            [1, QTS], F32, tag="rcf", name=f"rcf{b}_{h}_{qt}"
                        )
                        nc.vector.reciprocal_approx_fast(rcf[:], dsb[:])
                        rcr = rcp.tile(
                            [1, QTS], F16, tag="rcr", name=f"rcr{b}_{h}_{qt}"
                        )
                        nc.vector.tensor_copy(rcr[:], rcf[:])
                        nc.tensor.matmul(
                            bc,
                            indh_t[:, h * 128 : (h + 1) * 128],
                            rcr[:],
                            start=(h == 0),
                            stop=(h == HPC - 1),
                        )
                    bc_sb = rcp.tile([128, QTS], F32, tag="bcsb", name=f"bc_sb{b}_{qt}")
                    nc.vector.tensor_copy(bc_sb[:], bc)
                    for h in range(HPC):
                        nc.vector.tensor_tensor(
                            ctxn[h * DH : (h + 1) * DH, :],
                            ctx_ps[(b, h)][0:DH, :],
                            bc_sb[h * DH : (h + 1) * DH, :],
                            mybir.AluOpType.mult,
                        )
                    for qs in range(QN):
                        ob = outp.tile([128, D], F16, tag="outb", name=f"ob{b}_{qs}_{qt}")
                        for oh in range(2):
                            op_ps = psp.tile(
                                [128, QTS], F32, tag="bank", name=f"op{b}_{qs}_{oh}_{qt}"
                            )
                            nc.tensor.matmul(
                                op_ps[:],
                                ctxn[:, qs * 128 : (qs + 1) * 128],
                                wo_t[:, oh * QTS : (oh + 1) * QTS],
                                start=True,
                                stop=True,
                            )
                            if oh == 0:
                                nc.vector.tensor_copy(
                                    ob[:, oh * QTS : (oh + 1) * QTS], op_ps[:]
                                )
                            else:
                                nc.scalar.copy(
                                    ob[:, oh * QTS : (oh + 1) * QTS], op_ps[:]
                                )
                        r0 = qt * QTS + qs * 128
                        nc.sync.dma_start(out[b, r0 : r0 + 128, :], ob[:])

            pending_epilogue = None
            for qt in range(QN):
                ctx_ps = {}
                pending_pv = []
                for kg in range(KGN):
                    if qt == 0 and kg == 0:
                        eb_t = pre_eb
                    else:
                        eb_t = ebp.tile(
                            [128, KTG, B, HPC, QTS], F16, tag="eb", name=f"eb_t{qt}_{kg}"
                        )
                        nc.scalar.dma_start(eb_t[:], eb[qt, kg])
                    for ki in range(KTG):
                        kt = kg * KTG + ki
                        # QK: 4 adjacent 64-row matmuls (h0 rows 0-63 /
                        # h1 rows 64-127 -> alternating PE row-tiles)
                        sc = {}
                        for b in range(B):
                            for h in range(HPC):
                                sc[(b, h)] = psp.tile(
                                    [128, QTS],
                                    F32,
                                    tag="bank",
                                    name=f"sc{b}_{h}_{kt}_{qt}",
                                )
                                nc.tensor.matmul(
                                    sc[(b, h)][:],
                                    hk_sb[b][
                                        h * DH : (h + 1) * DH, kt * KP : (kt + 1) * KP
                                    ],
                                    hq_sb[b][
                                        h * DH : (h + 1) * DH,
                                        qt * QTS : (qt + 1) * QTS,
                                    ],
                                    start=True,
                                    stop=True,
                                )
                        new_pv = []
                        for b in range(B):
                            for h in range(HPC):
                                p1_t = p1p.tile(
                                    [128, QTS],
                                    F16,
                                    tag="p1",
                                    name=f"p1_{b}_{h}_{kt}_{qt}",
                                )
                                nc.scalar.activation(
                                    p1_t[:],
                                    sc[(b, h)][:],
                                    mybir.ActivationFunctionType.Exp,
                                )
                                p_t = ptp.tile(
                                    [128, QTS],
                                    F16,
                                    tag="pt",
                                    name=f"p_t{b}_{h}_{kt}_{qt}",
                                )
                                nc.vector.tensor_tensor(
                                    p_t[:],
                                    p1_t[:],
                                    eb_t[:, ki, b, h, :],
                                    mybir.AluOpType.mult,
                                )
                                new_pv.append((b, h, kt, p_t))
                        # software pipeline: PV of previous k-tile (possibly
                        # from the previous q-tile, targeting its ctx banks)
                        for item in pending_pv:
                            if len(item) == 5:
                                pb_, ph_, pkt, p_t, tgt = item
                            else:
                                pb_, ph_, pkt, p_t = item
                                tgt = ctx_ps[(pb_, ph_)]
                            o = pkt * HVW + ph_ * (DH + 1)
                            nc.tensor.matmul(
                                tgt[:],
                                hv_sb[pb_][:, o : o + 128],
                                p_t[:],
                                start=(pkt == 0),
                                stop=(pkt == KN - 1),
                            )
                        pending_pv = new_pv
                        if kg == 0 and ki == 0:
                            # previous q-tile's normalize/out-proj lands here so
                            # the PE never drains at the boundary (keeps HAM
                            # warm); its ctx banks free up for this q-tile
                            if pending_epilogue is not None:
                                emit_epilogue(*pending_epilogue)
                                pending_epilogue = None
                            for bb in range(B):
                                for hh in range(HPC):
                                    ctx_ps[(bb, hh)] = psp.tile(
                                        [128, QTS],
                                        F32,
                                        tag="bank",
                                        name=f"ctx_ps{bb}_{hh}_{qt}",
                                    )
                for b, h, pkt, p_t in pending_pv:
                    o = pkt * HVW + h * (DH + 1)
                    nc.tensor.matmul(
                        ctx_ps[(b, h)][:],
                        hv_sb[b][:, o : o + 128],
                        p_t[:],
                        start=(pkt == 0),
                        stop=(pkt == KN - 1),
                    )
                pending_pv = []
                pending_epilogue = (qt, ctx_ps)
            emit_epilogue(*pending_epilogue)

    nc.compile()
    _CACHE["nc"] = nc
    return nc


def _prep_core(core, position_bias, Wq, Wk, Wv, Wo, shared):
    """Per-core input map. `shared` holds core-independent packed arrays."""
    h0 = core * HPC
    rows = slice(h0 * DH, (h0 + HPC) * DH)

    def packw(w, scale=1.0):
        return np.ascontiguousarray(
            (w[rows].T * scale).reshape(DC, 128, 128).transpose(1, 0, 2)
        ).astype(np.float16)

    # E = exp(pb) * keep, per (b, h) -> [qt, kg, kp, ktg, b, h, qf]
    ecomb = shared["epb"][h0 : h0 + HPC][None] * shared["keep"][:, None]  # [B,HPC,q,k]
    ebp = np.ascontiguousarray(
        ecomb.reshape(B, HPC, QN, QTS, KGN, KTG, 128).transpose(2, 4, 6, 5, 0, 1, 3)
    ).astype(np.float16)
    return {
        "qT": shared["qT"],
        "kvT": shared["kvT"],
        "identr": shared["identr"],
        "indh": shared["indh"],
        "wq": packw(Wq, 1.0 / np.sqrt(DH)),
        "wk": packw(Wk),
        "wv": packw(Wv),
        "wo": np.ascontiguousarray(Wo[:, rows].T).astype(np.float16),
        "eb": ebp,
    }


def _prep_shared(query, key_value, mask, position_bias):
    qTp = np.ascontiguousarray(
        query.reshape(B, L, DC, 128).transpose(0, 2, 3, 1)
    ).astype(np.float16)
    kvTp = np.ascontiguousarray(
        key_value.reshape(B, L, DC, 128).transpose(0, 2, 3, 1)
    ).astype(np.float16)
    epb = np.exp(position_bias, dtype=np.float32)  # [H, q, k]
    keep = np.asarray(mask, dtype=np.float32)  # [B, q, k] 1=keep
    indh = np.concatenate(
        [
            np.where(np.arange(128) < 64, 1.0, 0.0),
            np.where(np.arange(128) >= 64, 1.0, 0.0),
        ]
    ).astype(np.float16)[None, :]
    return {
        "qT": qTp,
        "kvT": kvTp,
        "epb": epb,
        "keep": keep,
        "identr": np.eye(128, dtype=np.float32),
        "indh": np.ascontiguousarray(indh),
    }


def kernel(query, key_value, mask, position_bias, Wq, Wk, Wv, Wo, _trace=False):
    query = np.asarray(query, dtype=np.float32)
    key_value = np.asarray(key_value, dtype=np.float32)
    mask = np.asarray(mask)
    position_bias = np.asarray(position_bias, dtype=np.float32)
    Wq = np.asarray(Wq, dtype=np.float32)
    Wk = np.asarray(Wk, dtype=np.float32)
    Wv = np.asarray(Wv, dtype=np.float32)
    Wo = np.asarray(Wo, dtype=np.float32)

    nc = _build()
    shared = _prep_shared(query, key_value, mask, position_bias)
    in_maps = [
        _prep_core(c, position_bias, Wq, Wk, Wv, Wo, shared) for c in range(N_CORES)
    ]
    res = run_bass_kernel_spmd(nc, in_maps, list(range(N_CORES)), trace=_trace)
    _CACHE["last_result"] = res
    acc = res.results[0]["out"].astype(np.float64)
    for c in range(1, N_CORES):
        acc += res.results[c]["out"]
    return acc.astype(np.float32)


# revision 5
# speedup vs baseline: 1.2266x; 1.2266x over previous
"""CPM3 attention kernel for 8 trn2 NeuronCores.

Sharding: tensor-parallel over heads (2 heads/core x both batches).
Device computes per-core partial outputs (Wo row-sharded); host sums.

Data layout tricks:
- host pre-transposes q/kv so the device never transposes big tensors;
  scores are computed transposed [k, q] so the softmax needs no
  partition-dim reductions (a ones-column in V yields the denominators).
- fp16 operands for all matmuls; PSUM accumulation stays fp32.
- softmax bias/mask enter MULTIPLICATIVELY: host precomputes
  E = exp(position_bias) * keep_mask, so p = exp(qk) * E.  This removes
  the per-tile identity matmul (PE) and scalar_tensor_tensor combine
  (DVE) of the additive formulation.
- QK scores for (h0, h1) land in one 2-bank PSUM tile, so a single
  ACT instruction exps 1024 columns (amortizes the ~350-cycle ACT
  instruction overhead).
- PV matmuls run 2 k-tiles behind QK so the PE never head-of-line
  blocks on the exp -> multiply chain.
- PSUM budget: tag "sc" = 2 tiles x 2 banks (score staging), tag "ctx"
  = 2 tiles x 2 banks (ctx accumulators, (b, h0/h1) packed per tile);
  prologue/epilogue tiles reuse the same rings.
"""

import sys

sys.path.insert(0, "/opt/trn_rl_repo")

import numpy as np
import ml_dtypes

import concourse.bass as bass
import concourse.bacc as bacc
import concourse.tile as tile
import concourse.mybir as mybir
from concourse.bass_utils import run_bass_kernel_spmd

B, L, D, H, DH = 2, 2048, 1024, 16, 64
N_CORES = 8
HPC = H // N_CORES  # heads per core = 2
QTS = 512  # q tile size
QN = L // QTS  # 4
KP = 128  # k partition tile
KN = L // KP  # 16
KTG = 4  # k tiles per DMA group
KGN = KN // KTG  # 4
DC = D // 128  # 8 contraction chunks
HVW = 2 * (DH + 1)  # 130: hv_aug columns per k-tile (2 heads x (64+ones))
PV_LAG = 2  # k-tiles of software-pipeline distance for PV

F32 = mybir.dt.float32
F32R = mybir.dt.float32r
F16 = mybir.dt.float16

_CACHE: dict = {}


def _build():
    if "nc" in _CACHE:
        return _CACHE["nc"]
    nc = bacc.Bacc("TRN2", target_bir_lowering=False, debug=False, num_devices=N_CORES)

    qT = nc.dram_tensor("qT", [B, DC, 128, L], F16, kind="ExternalInput").ap()
    kvT = nc.dram_tensor("kvT", [B, DC, 128, L], F16, kind="ExternalInput").ap()
    wq = nc.dram_tensor("wq", [128, DC, 128], F16, kind="ExternalInput").ap()
    wk = nc.dram_tensor("wk", [128, DC, 128], F16, kind="ExternalInput").ap()
    wv = nc.dram_tensor("wv", [128, DC, 128], F16, kind="ExternalInput").ap()
    wo = nc.dram_tensor("wo", [128, D], F16, kind="ExternalInput").ap()
    eb = nc.dram_tensor(
        "eb", [QN, KGN, 128, KTG, B, HPC, QTS], F16, kind="ExternalInput"
    ).ap()
    identr = nc.dram_tensor("identr", [128, 128], F32R, kind="ExternalInput").ap()
    indh = nc.dram_tensor("indh", [1, 256], F16, kind="ExternalInput").ap()
    out = nc.dram_tensor("out", [B, L, D], F16, kind="ExternalOutput").ap()

    with tile.TileContext(nc) as tc:
        with (
            tc.tile_pool(name="const", bufs=1) as constp,
            tc.tile_pool(name="hq", bufs=2) as hqp,
            tc.tile_pool(name="hk", bufs=2) as hkp,
            tc.tile_pool(name="hv", bufs=2) as hvp,
            tc.tile_pool(name="stage", bufs=3) as stagep,
            tc.tile_pool(name="ebp", bufs=3) as ebp,
            tc.tile_pool(name="p1", bufs=6) as p1p,
            tc.tile_pool(name="pt", bufs=14) as ptp,
            tc.tile_pool(name="ctxn", bufs=2) as ctxnp,
            tc.tile_pool(name="rc", bufs=4) as rcp,
            tc.tile_pool(name="outb", bufs=3) as outp,
            tc.tile_pool(name="psum", bufs=2, space=bass.MemorySpace.PSUM) as psp,
        ):
            # ---- constants ----
            identr_t = constp.tile([128, 128], F32R, tag="identr")
            nc.sync.dma_start(identr_t[:], identr[:])
            indh_t = constp.tile([1, 256], F16, tag="indh")
            nc.sync.dma_start(indh_t[:], indh[:])
            wq_t = constp.tile([128, DC, 128], F16, tag="wq")
            nc.sync.dma_start(wq_t[:], wq[:])
            wk_t = constp.tile([128, DC, 128], F16, tag="wk")
            nc.sync.dma_start(wk_t[:], wk[:])
            wv_t = constp.tile([128, DC, 128], F16, tag="wv")
            nc.sync.dma_start(wv_t[:], wv[:])
            wo_t = constp.tile([128, D], F16, tag="wo")
            nc.sync.dma_start(wo_t[:], wo[:])

            # ---- prefetch first E group (overlaps prologue) ----
            pre_eb = ebp.tile([128, KTG, B, HPC, QTS], F16, tag="eb", name="pre_eb")
            nc.gpsimd.dma_start(pre_eb[:], eb[0, 0])

            # ---- prologue: projections ----
            # PSUM tiles are [128, 2, QTS] (2 banks); qt pairs share a tile.
            hq_sb, hk_sb, hv_sb = {}, {}, {}
            for b in range(B):
                hq_sb[b] = hqp.tile([128, L], F16, tag="hq", name=f"hq_sb{b}")
                hq_ps = [
                    psp.tile([128, 2, QTS], F32, tag="sc", name=f"hq_ps{b}_{i}")
                    for i in range(2)
                ]
                for dc in range(DC):
                    qc = stagep.tile([128, L], F16, tag="stage")
                    nc.sync.dma_start(qc[:], qT[b, dc])
                    for qt in range(QN):
                        nc.tensor.matmul(
                            hq_ps[qt // 2][:, qt % 2, :],
                            wq_t[:, dc, :],
                            qc[:, qt * QTS : (qt + 1) * QTS],
                            start=(dc == 0),
                            stop=(dc == DC - 1),
                        )
                for qt in range(QN):
                    nc.scalar.copy(
                        hq_sb[b][:, qt * QTS : (qt + 1) * QTS],
                        hq_ps[qt // 2][:, qt % 2, :],
                    )

                hk_sb[b] = hkp.tile([128, L], F16, tag="hk", name=f"hk_sb{b}")
                hvT = stagep.tile([128, L], F32R, tag="stage")
                hk_ps = [
                    psp.tile([128, 2, QTS], F32, tag="sc", name=f"hk_ps{b}_{i}")
                    for i in range(2)
                ]
                hv_ps = [
                    psp.tile([128, 2, QTS], F32, tag="ctx", name=f"hv_ps{b}_{i}")
                    for i in range(2)
                ]
                for dc in range(DC):
                    kc = stagep.tile([128, L], F16, tag="stage")
                    nc.sync.dma_start(kc[:], kvT[b, dc])
                    for qt in range(QN):
                        nc.tensor.matmul(
                            hk_ps[qt // 2][:, qt % 2, :],
                            wk_t[:, dc, :],
                            kc[:, qt * QTS : (qt + 1) * QTS],
                            start=(dc == 0),
                            stop=(dc == DC - 1),
                        )
                        nc.tensor.matmul(
                            hv_ps[qt // 2][:, qt % 2, :],
                            wv_t[:, dc, :],
                            kc[:, qt * QTS : (qt + 1) * QTS],
                            start=(dc == 0),
                            stop=(dc == DC - 1),
                        )
                for qt in range(QN):
                    nc.scalar.copy(
                        hk_sb[b][:, qt * QTS : (qt + 1) * QTS],
                        hk_ps[qt // 2][:, qt % 2, :],
                    )
                    nc.vector.tensor_copy(
                        hvT[:, qt * QTS : (qt + 1) * QTS],
                        hv_ps[qt // 2][:, qt % 2, :],
                    )

                # hv_aug: transpose hvT per k-tile; ones cols prefilled
                hv_sb[b] = hvp.tile([128, KN * HVW + 64], F16, tag="hv", name=f"hv_sb{b}")
                nc.gpsimd.memset(hv_sb[b][:].bitcast(mybir.dt.uint16), 0x3C00)
                for kt in range(KN):
                    tp = psp.tile([128, 128], F32R, tag="sc")
                    nc.tensor.transpose(
                        tp[:], hvT[:, kt * KP : (kt + 1) * KP], identr_t[:]
                    )
                    o = kt * HVW
                    nc.vector.tensor_copy(hv_sb[b][:, o : o + DH], tp[:, 0:DH])
                    nc.vector.tensor_copy(
                        hv_sb[b][:, o + DH + 1 : o + 2 * DH + 1], tp[:, DH:128]
                    )

            # ---- epilogue helper ----
            def emit_epilogue(qt, ctx_ps):
                # normalize + output projection for a finished q-tile
                # ctx_ps[b] is a [128, HPC, QTS] psum tile (h in dim 1)
                for b in range(B):
                    ctxn = ctxnp.tile(
                        [128, QTS], F16, tag="ctxn", name=f"ctxn{b}_{qt}"
                    )
                    bcw = psp.tile([128, 2, QTS], F32, tag="sc", name=f"bcw{b}_{qt}")
                    bc = bcw[:, 0, :]
                    for h in range(HPC):
                        dsb = rcp.tile(
                            [1, QTS], F32, tag="dsb", name=f"dsb{b}_{h}_{qt}"
                        )
                        nc.vector.tensor_copy(dsb[:], ctx_ps[b][DH : DH + 1, h, :])
                        rcf = rcp.tile(
                            [1, QTS], F32, tag="rcf", name=f"rcf{b}_{h}_{qt}"
                        )
                        nc.vector.reciprocal_approx_fast(rcf[:], dsb[:])
                        rcr = rcp.tile(
                            [1, QTS], F16, tag="rcr", name=f"rcr{b}_{h}_{qt}"
                        )
                        nc.vector.tensor_copy(rcr[:], rcf[:])
                        nc.tensor.matmul(
                            bc,
                            indh_t[:, h * 128 : (h + 1) * 128],
                            rcr[:],
                            start=(h == 0),
                            stop=(h == HPC - 1),
                        )
                    bc_sb = rcp.tile([128, QTS], F32, tag="bcsb", name=f"bc_sb{b}_{qt}")
                    nc.vector.tensor_copy(bc_sb[:], bc)
                    for h in range(HPC):
                        nc.vector.tensor_tensor(
                            ctxn[h * DH : (h + 1) * DH, :],
                            ctx_ps[b][0:DH, h, :],
                            bc_sb[h * DH : (h + 1) * DH, :],
                            mybir.AluOpType.mult,
                        )
                    for qs in range(QN):
                        ob = outp.tile([128, D], F16, tag="outb", name=f"ob{b}_{qs}_{qt}")
                        op_ps = psp.tile(
                            [128, 2, QTS], F32, tag="sc", name=f"op{b}_{qs}_{qt}"
                        )
                        for oh in range(2):
                            nc.tensor.matmul(
                                op_ps[:, oh, :],
                                ctxn[:, qs * 128 : (qs + 1) * 128],
                                wo_t[:, oh * QTS : (oh + 1) * QTS],
                                start=True,
                                stop=True,
                            )
                            if oh == 0:
                                nc.vector.tensor_copy(
                                    ob[:, oh * QTS : (oh + 1) * QTS], op_ps[:, oh, :]
                                )
                            else:
                                nc.scalar.copy(
                                    ob[:, oh * QTS : (oh + 1) * QTS], op_ps[:, oh, :]
                                )
                        r0 = qt * QTS + qs * 128
                        nc.sync.dma_start(out[b, r0 : r0 + 128, :], ob[:])

            # ---- main loop ----
            ctx_map = {}  # qt -> {b: [128, HPC, QTS] psum ap}
            pending_pv = []  # groups of [(b, h, kt, p_t, qt)], oldest first
            pending_epilogue = None

            def flush_pv_group():
                group = pending_pv.pop(0)
                for b, h, pkt, p_t, pqt in group:
                    o = pkt * HVW + h * (DH + 1)
                    nc.tensor.matmul(
                        ctx_map[pqt][b][:, h, :],
                        hv_sb[b][:, o : o + 128],
                        p_t[:],
                        start=(pkt == 0),
                        stop=(pkt == KN - 1),
                    )

            for qt in range(QN):
                for kg in range(KGN):
                    if qt == 0 and kg == 0:
                        eb_t = pre_eb
                    else:
                        eb_t = ebp.tile(
                            [128, KTG, B, HPC, QTS], F16, tag="eb", name=f"eb_t{qt}_{kg}"
                        )
                        nc.gpsimd.dma_start(eb_t[:], eb[qt, kg])
                    for ki in range(KTG):
                        kt = kg * KTG + ki
                        if kg == 0 and ki == PV_LAG and pending_epilogue is not None:
                            # all of qt-1's PVs have flushed; retire it (frees
                            # its ctx banks for this q-tile's accumulators)
                            emit_epilogue(*pending_epilogue)
                            pending_epilogue = None
                        if kg == 0 and ki == 0:
                            ctx_map[qt] = {
                                bb: psp.tile(
                                    [128, HPC, QTS],
                                    F32,
                                    tag="ctx",
                                    name=f"ctx_ps{bb}_{qt}",
                                )
                                for bb in range(B)
                            }
                        # QK: (h0, h1) into a 2-bank PSUM tile (h0 rows 0-63 /
                        # h1 rows 64-127 also land on different PE row-tiles)
                        sc = {}
                        for b in range(B):
                            sc[b] = psp.tile(
                                [128, HPC, QTS],
                                F32,
                                tag="sc",
                                name=f"sc{b}_{kt}_{qt}",
                            )
                            for h in range(HPC):
                                nc.tensor.matmul(
                                    sc[b][:, h, :],
                                    hk_sb[b][
                                        h * DH : (h + 1) * DH, kt * KP : (kt + 1) * KP
                                    ],
                                    hq_sb[b][
                                        h * DH : (h + 1) * DH,
                                        qt * QTS : (qt + 1) * QTS,
                                    ],
                                    start=True,
                                    stop=True,
                                )
                        new_group = []
                        for b in range(B):
                            p1_t = p1p.tile(
                                [128, HPC, QTS], F16, tag="p1", name=f"p1_{b}_{kt}_{qt}"
                            )
                            nc.scalar.activation(
                                p1_t[:],
                                sc[b][:],
                                mybir.ActivationFunctionType.Exp,
                            )
                            for h in range(HPC):
                                p_t = ptp.tile(
                                    [128, QTS],
                                    F16,
                                    tag="pt",
                                    name=f"p_t{b}_{h}_{kt}_{qt}",
                                )
                                nc.vector.tensor_tensor(
                                    p_t[:],
                                    p1_t[:, h, :],
                                    eb_t[:, ki, b, h, :],
                                    mybir.AluOpType.mult,
                                )
                                new_group.append((b, h, kt, p_t, qt))
                        pending_pv.append(new_group)
                        if len(pending_pv) > PV_LAG:
                            flush_pv_group()
                pending_epilogue = (qt, ctx_map[qt])
            while pending_pv:
                flush_pv_group()
            emit_epilogue(*pending_epilogue)

    nc.compile()
    _CACHE["nc"] = nc
    return nc


def _prep_core(core, position_bias, Wq, Wk, Wv, Wo, shared):
    """Per-core input map. `shared` holds core-independent packed arrays."""
    h0 = core * HPC
    rows = slice(h0 * DH, (h0 + HPC) * DH)

    def packw(w, scale=1.0):
        return np.ascontiguousarray(
            (w[rows].T * scale).reshape(DC, 128, 128).transpose(1, 0, 2)
        ).astype(np.float16)

    # E = exp(pb) * keep, per (b, h) -> [qt, kg, kp, ktg, b, h, qf]
    ecomb = shared["epb"][h0 : h0 + HPC][None] * shared["keep"][:, None]  # [B,HPC,q,k]
    ebp = np.ascontiguousarray(
        ecomb.reshape(B, HPC, QN, QTS, KGN, KTG, 128).transpose(2, 4, 6, 5, 0, 1, 3)
    ).astype(np.float16)
    return {
        "qT": shared["qT"],
        "kvT": shared["kvT"],
        "identr": shared["identr"],
        "indh": shared["indh"],
        "wq": packw(Wq, 1.0 / np.sqrt(DH)),
        "wk": packw(Wk),
        "wv": packw(Wv),
        "wo": np.ascontiguousarray(Wo[:, rows].T).astype(np.float16),
        "eb": ebp,
    }


def _prep_shared(query, key_value, mask, position_bias):
    qTp = np.ascontiguousarray(
        query.reshape(B, L, DC, 128).transpose(0, 2, 3, 1)
    ).astype(np.float16)
    kvTp = np.ascontiguousarray(
        key_value.reshape(B, L, DC, 128).transpose(0, 2, 3, 1)
    ).astype(np.float16)
    epb = np.exp(position_bias, dtype=np.float32)  # [H, q, k]
    keep = np.asarray(mask, dtype=np.float32)  # [B, q, k] 1=keep
    indh = np.concatenate(
        [
            np.where(np.arange(128) < 64, 1.0, 0.0),
            np.where(np.arange(128) >= 64, 1.0, 0.0),
        ]
    ).astype(np.float16)[None, :]
    return {
        "qT": qTp,
        "kvT": kvTp,
        "epb": epb,
        "keep": keep,
        "identr": np.eye(128, dtype=np.float32),
        "indh": np.ascontiguousarray(indh),
    }


def kernel(query, key_value, mask, position_bias, Wq, Wk, Wv, Wo, _trace=False):
    query = np.asarray(query, dtype=np.float32)
    key_value = np.asarray(key_value, dtype=np.float32)
    mask = np.asarray(mask)
    position_bias = np.asarray(position_bias, dtype=np.float32)
    Wq = np.asarray(Wq, dtype=np.float32)
    Wk = np.asarray(Wk, dtype=np.float32)
    Wv = np.asarray(Wv, dtype=np.float32)
    Wo = np.asarray(Wo, dtype=np.float32)

    nc = _build()
    shared = _prep_shared(query, key_value, mask, position_bias)
    in_maps = [
        _prep_core(c, position_bias, Wq, Wk, Wv, Wo, shared) for c in range(N_CORES)
    ]
    res = run_bass_kernel_spmd(nc, in_maps, list(range(N_CORES)), trace=_trace)
    _CACHE["last_result"] = res
    acc = res.results[0]["out"].astype(np.float64)
    for c in range(1, N_CORES):
        acc += res.results[c]["out"]
    return acc.astype(np.float32)


# revision 6
# speedup vs baseline: 1.2633x; 1.0299x over previous
"""CPM3 attention kernel for 8 trn2 NeuronCores.

Sharding: tensor-parallel over heads (2 heads/core x both batches).
Device computes per-core partial outputs (Wo row-sharded); host sums.

Data layout tricks:
- host pre-transposes q/kv so the device never transposes big tensors;
  scores are computed transposed [k, q] so the softmax needs no
  partition-dim reductions (a ones-column in V yields the denominators).
- fp16 operands for all matmuls; PSUM accumulation stays fp32.
- softmax bias/mask enter MULTIPLICATIVELY: host precomputes
  E = exp(position_bias) * keep_mask, so p = exp(qk) * E.
- QK scores for (h0, h1) land in one 2-bank PSUM tile, so a single
  ACT instruction exps 1024 columns (amortizes ACT instruction
  overhead); PV runs 2 k-tiles behind QK (software pipeline).
- epilogue split: only the softmax normalization happens at q-tile
  boundaries; all output projections run as one dense tail block so
  they never stall the main-loop pipeline.
- PSUM budget: tag "sc" = 2 tiles x 2 banks, tag "ctx" = 2 tiles x 2
  banks; prologue/epilogue tiles reuse the same rings.
"""

import sys

sys.path.insert(0, "/opt/trn_rl_repo")

import numpy as np
import ml_dtypes

import concourse.bass as bass
import concourse.bacc as bacc
import concourse.tile as tile
import concourse.mybir as mybir
from concourse.bass_utils import run_bass_kernel_spmd

B, L, D, H, DH = 2, 2048, 1024, 16, 64
N_CORES = 8
HPC = H // N_CORES  # heads per core = 2
QTS = 512  # q tile size
QN = L // QTS  # 4
KP = 128  # k partition tile
KN = L // KP  # 16
KTG = 4  # k tiles per DMA group
KGN = KN // KTG  # 4
DC = D // 128  # 8 contraction chunks
HVW = 2 * (DH + 1)  # 130: hv_aug columns per k-tile (2 heads x (64+ones))
PV_LAG = 2  # k-tiles of software-pipeline distance for PV

F32 = mybir.dt.float32
F32R = mybir.dt.float32r
F16 = mybir.dt.float16

_CACHE: dict = {}


def _build():
    if "nc" in _CACHE:
        return _CACHE["nc"]
    nc = bacc.Bacc("TRN2", target_bir_lowering=False, debug=False, num_devices=N_CORES)

    qT = nc.dram_tensor("qT", [B, DC, 128, L], F16, kind="ExternalInput").ap()
    kvT = nc.dram_tensor("kvT", [B, DC, 128, L], F16, kind="ExternalInput").ap()
    wq = nc.dram_tensor("wq", [128, DC, 128], F16, kind="ExternalInput").ap()
    wk = nc.dram_tensor("wk", [128, DC, 128], F16, kind="ExternalInput").ap()
    wv = nc.dram_tensor("wv", [128, DC, 128], F16, kind="ExternalInput").ap()
    wo = nc.dram_tensor("wo", [128, D], F16, kind="ExternalInput").ap()
    eb = nc.dram_tensor(
        "eb", [QN, KGN, 128, KTG, B, HPC, QTS], F16, kind="ExternalInput"
    ).ap()
    identr = nc.dram_tensor("identr", [128, 128], F32R, kind="ExternalInput").ap()
    indh = nc.dram_tensor("indh", [1, 256], F16, kind="ExternalInput").ap()
    out = nc.dram_tensor("out", [B, L, D], F16, kind="ExternalOutput").ap()

    with tile.TileContext(nc) as tc:
        with (
            tc.tile_pool(name="const", bufs=1) as constp,
            tc.tile_pool(name="hq", bufs=2) as hqp,
            tc.tile_pool(name="hk", bufs=2) as hkp,
            tc.tile_pool(name="hv", bufs=2) as hvp,
            tc.tile_pool(name="stage", bufs=3) as stagep,
            tc.tile_pool(name="ebp", bufs=2) as ebp,
            tc.tile_pool(name="p1", bufs=6) as p1p,
            tc.tile_pool(name="pt", bufs=14) as ptp,
            tc.tile_pool(name="ctxn", bufs=2 * QN) as ctxnp,
            tc.tile_pool(name="rc", bufs=4) as rcp,
            tc.tile_pool(name="outb", bufs=4) as outp,
            tc.tile_pool(name="psum", bufs=2, space=bass.MemorySpace.PSUM) as psp,
        ):
            # ---- constants (wq/wk first: the prologue blocks on them) ----
            wq_t = constp.tile([128, DC, 128], F16, tag="wq")
            nc.sync.dma_start(wq_t[:], wq[:])
            wk_t = constp.tile([128, DC, 128], F16, tag="wk")
            nc.scalar.dma_start(wk_t[:], wk[:])
            wv_t = constp.tile([128, DC, 128], F16, tag="wv")
            nc.scalar.dma_start(wv_t[:], wv[:])
            identr_t = constp.tile([128, 128], F32R, tag="identr")
            nc.sync.dma_start(identr_t[:], identr[:])
            indh_t = constp.tile([1, 256], F16, tag="indh")
            nc.sync.dma_start(indh_t[:], indh[:])
            wo_t = constp.tile([128, D], F16, tag="wo")
            nc.sync.dma_start(wo_t[:], wo[:])

            # ---- prologue: projections ----
            # q chunks on the sync queue, kv chunks on the scalar queue (more
            # SDMA parallelism); each [128, L] chunk is loaded as two halves
            # so matmuls start after the first half arrives.
            hq_sb, hk_sb, hv_sb, hvT = {}, {}, {}, {}
            hq_ps, hk_ps, hv_ps = {}, {}, {}

            def half_dma(engine, dst, src):
                engine.dma_start(dst[:, 0 : L // 2], src[:, 0 : L // 2])
                engine.dma_start(dst[:, L // 2 : L], src[:, L // 2 : L])

            for b in range(B):
                # -- projection matmuls for batch b --
                hq_ps[b] = [
                    psp.tile([128, 2, QTS], F32, tag="sc", name=f"hq_ps{b}_{i}")
                    for i in range(2)
                ]
                for dc in range(DC):
                    qc = stagep.tile([128, L], F16, tag="stage", name=f"qc{b}_{dc}")
                    half_dma(nc.sync, qc, qT[b, dc])
                    for qt in range(QN):
                        nc.tensor.matmul(
                            hq_ps[b][qt // 2][:, qt % 2, :],
                            wq_t[:, dc, :],
                            qc[:, qt * QTS : (qt + 1) * QTS],
                            start=(dc == 0),
                            stop=(dc == DC - 1),
                        )
                hq_sb[b] = hqp.tile([128, L], F16, tag="hq", name=f"hq_sb{b}")
                for qt in range(QN):
                    nc.scalar.copy(
                        hq_sb[b][:, qt * QTS : (qt + 1) * QTS],
                        hq_ps[b][qt // 2][:, qt % 2, :],
                    )

                hk_ps[b] = [
                    psp.tile([128, 2, QTS], F32, tag="sc", name=f"hk_ps{b}_{i}")
                    for i in range(2)
                ]
                hv_ps[b] = [
                    psp.tile([128, 2, QTS], F32, tag="ctx", name=f"hv_ps{b}_{i}")
                    for i in range(2)
                ]
                for dc in range(DC):
                    kc = stagep.tile([128, L], F16, tag="stage", name=f"kc{b}_{dc}")
                    half_dma(nc.scalar, kc, kvT[b, dc])
                    for qt in range(QN):
                        nc.tensor.matmul(
                            hk_ps[b][qt // 2][:, qt % 2, :],
                            wk_t[:, dc, :],
                            kc[:, qt * QTS : (qt + 1) * QTS],
                            start=(dc == 0),
                            stop=(dc == DC - 1),
                        )
                        nc.tensor.matmul(
                            hv_ps[b][qt // 2][:, qt % 2, :],
                            wv_t[:, dc, :],
                            kc[:, qt * QTS : (qt + 1) * QTS],
                            start=(dc == 0),
                            stop=(dc == DC - 1),
                        )
                hk_sb[b] = hkp.tile([128, L], F16, tag="hk", name=f"hk_sb{b}")
                hvT[b] = stagep.tile([128, L], F32R, tag="stage", name=f"hvT{b}")
                for qt in range(QN):
                    nc.scalar.copy(
                        hk_sb[b][:, qt * QTS : (qt + 1) * QTS],
                        hk_ps[b][qt // 2][:, qt % 2, :],
                    )
                    nc.vector.tensor_copy(
                        hvT[b][:, qt * QTS : (qt + 1) * QTS],
                        hv_ps[b][qt // 2][:, qt % 2, :],
                    )

            # E prefetch: issued only now so the projection DMAs get the
            # full HBM bandwidth first
            pre_eb = ebp.tile([128, KTG, B, HPC, QTS], F16, tag="eb", name="pre_eb")
            nc.gpsimd.dma_start(pre_eb[:], eb[0, 0])

            # -- hv_aug: transpose hvT per k-tile; ones cols prefilled --
            for b in range(B):
                hv_sb[b] = hvp.tile(
                    [128, KN * HVW + 64], F16, tag="hv", name=f"hv_sb{b}"
                )
                nc.gpsimd.memset(hv_sb[b][:].bitcast(mybir.dt.uint16), 0x3C00)
            for b in range(B):
                for kt in range(KN):
                    tp = psp.tile([128, 128], F32R, tag="sc")
                    nc.tensor.transpose(
                        tp[:], hvT[b][:, kt * KP : (kt + 1) * KP], identr_t[:]
                    )
                    o = kt * HVW
                    nc.vector.tensor_copy(hv_sb[b][:, o : o + DH], tp[:, 0:DH])
                    nc.vector.tensor_copy(
                        hv_sb[b][:, o + DH + 1 : o + 2 * DH + 1], tp[:, DH:128]
                    )

            # ---- normalize: softmax denominators -> ctxn (fp16) ----
            ctxn_sb = {}

            def emit_normalize(qt, ctx_ps):
                for b in range(B):
                    ctxn = ctxnp.tile(
                        [128, QTS], F16, tag="ctxn", name=f"ctxn{b}_{qt}"
                    )
                    ctxn_sb[(b, qt)] = ctxn
                    bcw = psp.tile([128, 2, QTS], F32, tag="sc", name=f"bcw{b}_{qt}")
                    bc = bcw[:, 0, :]
                    for h in range(HPC):
                        dsb = rcp.tile(
                            [1, QTS], F32, tag="dsb", name=f"dsb{b}_{h}_{qt}"
                        )
                        nc.vector.tensor_copy(dsb[:], ctx_ps[b][DH : DH + 1, h, :])
                        rcf = rcp.tile(
                            [1, QTS], F32, tag="rcf", name=f"rcf{b}_{h}_{qt}"
                        )
                        nc.vector.reciprocal_approx_fast(rcf[:], dsb[:])
                        rcr = rcp.tile(
                            [1, QTS], F16, tag="rcr", name=f"rcr{b}_{h}_{qt}"
                        )
                        nc.vector.tensor_copy(rcr[:], rcf[:])
                        nc.tensor.matmul(
                            bc,
                            indh_t[:, h * 128 : (h + 1) * 128],
                            rcr[:],
                            start=(h == 0),
                            stop=(h == HPC - 1),
                        )
                    bc_sb = rcp.tile([128, QTS], F32, tag="bcsb", name=f"bc_sb{b}_{qt}")
                    nc.vector.tensor_copy(bc_sb[:], bc)
                    for h in range(HPC):
                        nc.vector.tensor_tensor(
                            ctxn[h * DH : (h + 1) * DH, :],
                            ctx_ps[b][0:DH, h, :],
                            bc_sb[h * DH : (h + 1) * DH, :],
                            mybir.AluOpType.mult,
                        )

            # ---- main loop ----
            ctx_map = {}  # qt -> {b: [128, HPC, QTS] psum ap}
            pending_pv = []  # groups of [(b, h, kt, p_t, qt)], oldest first
            pending_norm = None

            def flush_pv_group():
                group = pending_pv.pop(0)
                for b, h, pkt, p_t, pqt in group:
                    o = pkt * HVW + h * (DH + 1)
                    nc.tensor.matmul(
                        ctx_map[pqt][b][:, h, :],
                        hv_sb[b][:, o : o + 128],
                        p_t[:],
                        start=(pkt == 0),
                        stop=(pkt == KN - 1),
                    )

            for qt in range(QN):
                for kg in range(KGN):
                    if qt == 0 and kg == 0:
                        eb_t = pre_eb
                    else:
                        eb_t = ebp.tile(
                            [128, KTG, B, HPC, QTS], F16, tag="eb", name=f"eb_t{qt}_{kg}"
                        )
                        nc.gpsimd.dma_start(eb_t[:], eb[qt, kg])
                    for ki in range(KTG):
                        kt = kg * KTG + ki
                        if kg == 0 and ki == PV_LAG and pending_norm is not None:
                            # all of qt-1's PVs have flushed; normalize it
                            # (frees its ctx banks for this q-tile)
                            emit_normalize(*pending_norm)
                            pending_norm = None
                        if kg == 0 and ki == 0:
                            ctx_map[qt] = {
                                bb: psp.tile(
                                    [128, HPC, QTS],
                                    F32,
                                    tag="ctx",
                                    name=f"ctx_ps{bb}_{qt}",
                                )
                                for bb in range(B)
                            }
                        # QK: (h0, h1) into a 2-bank PSUM tile (h0 rows 0-63 /
                        # h1 rows 64-127 also land on different PE row-tiles)
                        sc = {}
                        for b in range(B):
                            sc[b] = psp.tile(
                                [128, HPC, QTS],
                                F32,
                                tag="sc",
                                name=f"sc{b}_{kt}_{qt}",
                            )
                            for h in range(HPC):
                                nc.tensor.matmul(
                                    sc[b][:, h, :],
                                    hk_sb[b][
                                        h * DH : (h + 1) * DH, kt * KP : (kt + 1) * KP
                                    ],
                                    hq_sb[b][
                                        h * DH : (h + 1) * DH,
                                        qt * QTS : (qt + 1) * QTS,
                                    ],
                                    start=True,
                                    stop=True,
                                )
                        new_group = []
                        for b in range(B):
                            p1_t = p1p.tile(
                                [128, HPC, QTS], F16, tag="p1", name=f"p1_{b}_{kt}_{qt}"
                            )
                            nc.scalar.activation(
                                p1_t[:],
                                sc[b][:],
                                mybir.ActivationFunctionType.Exp,
                            )
                            for h in range(HPC):
                                p_t = ptp.tile(
                                    [128, QTS],
                                    F16,
                                    tag="pt",
                                    name=f"p_t{b}_{h}_{kt}_{qt}",
                                )
                                nc.vector.tensor_tensor(
                                    p_t[:],
                                    p1_t[:, h, :],
                                    eb_t[:, ki, b, h, :],
                                    mybir.AluOpType.mult,
                                )
                                new_group.append((b, h, kt, p_t, qt))
                        pending_pv.append(new_group)
                        if len(pending_pv) > PV_LAG:
                            flush_pv_group()
                pending_norm = (qt, ctx_map[qt])
            while pending_pv:
                flush_pv_group()
            emit_normalize(*pending_norm)

            # ---- tail: dense output-projection block ----
            for qt in range(QN):
                for b in range(B):
                    ctxn = ctxn_sb[(b, qt)]
                    for qs in range(QN):
                        ob = outp.tile(
                            [128, D], F16, tag="outb", name=f"ob{b}_{qs}_{qt}"
                        )
                        op_ps = psp.tile(
                            [128, 2, QTS], F32, tag="sc", name=f"op{b}_{qs}_{qt}"
                        )
                        for oh in range(2):
                            nc.tensor.matmul(
                                op_ps[:, oh, :],
                                ctxn[:, qs * 128 : (qs + 1) * 128],
                                wo_t[:, oh * QTS : (oh + 1) * QTS],
                                start=True,
                                stop=True,
                            )
                            if oh == 0:
                                nc.vector.tensor_copy(
                                    ob[:, oh * QTS : (oh + 1) * QTS], op_ps[:, oh, :]
                                )
                            else:
                                nc.scalar.copy(
                                    ob[:, oh * QTS : (oh + 1) * QTS], op_ps[:, oh, :]
                                )
                        r0 = qt * QTS + qs * 128
                        dq = nc.sync if (qs % 2 == 0) else nc.scalar
                        dq.dma_start(out[b, r0 : r0 + 128, :], ob[:])

    nc.compile()
    _CACHE["nc"] = nc
    return nc


def _prep_core(core, position_bias, Wq, Wk, Wv, Wo, shared):
    """Per-core input map. `shared` holds core-independent packed arrays."""
    h0 = core * HPC
    rows = slice(h0 * DH, (h0 + HPC) * DH)

    def packw(w, scale=1.0):
        return np.ascontiguousarray(
            (w[rows].T * scale).reshape(DC, 128, 128).transpose(1, 0, 2)
        ).astype(np.float16)

    # E = exp(pb) * keep, per (b, h) -> [qt, kg, kp, ktg, b, h, qf]
    ecomb = shared["epb"][h0 : h0 + HPC][None] * shared["keep"][:, None]  # [B,HPC,q,k]
    ebp = np.ascontiguousarray(
        ecomb.reshape(B, HPC, QN, QTS, KGN, KTG, 128).transpose(2, 4, 6, 5, 0, 1, 3)
    ).astype(np.float16)
    return {
        "qT": shared["qT"],
        "kvT": shared["kvT"],
        "identr": shared["identr"],
        "indh": shared["indh"],
        "wq": packw(Wq, 1.0 / np.sqrt(DH)),
        "wk": packw(Wk),
        "wv": packw(Wv),
        "wo": np.ascontiguousarray(Wo[:, rows].T).astype(np.float16),
        "eb": ebp,
    }


def _prep_shared(query, key_value, mask, position_bias):
    qTp = np.ascontiguousarray(
        query.reshape(B, L, DC, 128).transpose(0, 2, 3, 1)
    ).astype(np.float16)
    kvTp = np.ascontiguousarray(
        key_value.reshape(B, L, DC, 128).transpose(0, 2, 3, 1)
    ).astype(np.float16)
    epb = np.exp(position_bias, dtype=np.float32)  # [H, q, k]
    keep = np.asarray(mask, dtype=np.float32)  # [B, q, k] 1=keep
    indh = np.concatenate(
        [
            np.where(np.arange(128) < 64, 1.0, 0.0),
            np.where(np.arange(128) >= 64, 1.0, 0.0),
        ]
    ).astype(np.float16)[None, :]
    return {
        "qT": qTp,
        "kvT": kvTp,
        "epb": epb,
        "keep": keep,
        "identr": np.eye(128, dtype=np.float32),
        "indh": np.ascontiguousarray(indh),
    }


def kernel(query, key_value, mask, position_bias, Wq, Wk, Wv, Wo, _trace=False):
    query = np.asarray(query, dtype=np.float32)
    key_value = np.asarray(key_value, dtype=np.float32)
    mask = np.asarray(mask)
    position_bias = np.asarray(position_bias, dtype=np.float32)
    Wq = np.asarray(Wq, dtype=np.float32)
    Wk = np.asarray(Wk, dtype=np.float32)
    Wv = np.asarray(Wv, dtype=np.float32)
    Wo = np.asarray(Wo, dtype=np.float32)

    nc = _build()
    shared = _prep_shared(query, key_value, mask, position_bias)
    in_maps = [
        _prep_core(c, position_bias, Wq, Wk, Wv, Wo, shared) for c in range(N_CORES)
    ]
    res = run_bass_kernel_spmd(nc, in_maps, list(range(N_CORES)), trace=_trace)
    _CACHE["last_result"] = res
    acc = res.results[0]["out"].astype(np.float64)
    for c in range(1, N_CORES):
        acc += res.results[c]["out"]
    return acc.astype(np.float32)
